# revision 20
# baseline (speedup 1.0000x reference)
"""Trainium2 Bass kernel for nn_DeConvAfterDownSampling.

Math (from the reference): with s[n] = sum_w x[b,c,h,w] flattened over
n = (b,c,h), Wf = W.reshape(F, P):

    out[0, f, n, p] = relu(s[n] * Wf[f, p] + b[f])      # (1, F, N, P)

N = 8*64*64 = 32768, F = 64, P = 25.  Output is ~210 MB fp32 while inputs
are ~8 MB, so the kernel is bound by the output HBM write.

Sharding: data-parallel over n across 8 cores (N_LOCAL = 4096 per core);
W and b replicated; no cross-core communication.

Per-core plan (partitions = (h, f) with h in {0,1} stacking two n-halves
so all 128 partitions are used):
  1. One DMA loads x (4096, 64) into SBUF as (128, 32, 64), partition
     i <- row 128*t + i.
  2. PE transposes each (128 n, 64 w) block -> (64 w, 128 n) in PSUM;
     copies assemble xT (128=(h,w), 512 n) in SBUF.
  3. One K=128 matmul with a constant block-diagonal ones matrix E
     (E[(h',w),(h,f)] = (h==h')) reduces over w and broadcasts:
     s_bcast[(h,f), j] = s[tile_base + 512h + j] for every f.
  4. For each p in 0..24 one elementwise op computes
     relu(W[f,p] * s + b[f]) with W[:,p] as per-partition scale and b as
     per-partition bias, writing the (stride 25) p-slice of the output
     tile.  Ops are split across ScalarE (activation, reads PSUM) and
     VectorE (tensor_scalar mult+max, reads an SBUF copy).
  5. One ~6.5 MB DMA per tile writes the (128, 512, 25) tile to HBM; the
     per-partition free layout (n-major, p-minor) is exactly contiguous
     HBM order, so each partition is a single 51.2 KB contiguous chunk.
"""

import numpy as np

import concourse.bass as bass
import concourse.mybir as mybir
from concourse import bacc, masks, tile
from concourse.bass_utils import run_bass_kernel_spmd

F32 = mybir.dt.float32

N_CORES = 8
B, C, H, WDIM = 8, 64, 64, 64
F, P = 64, 25
N_TOTAL = B * C * H          # 32768
N_LOCAL = N_TOTAL // N_CORES  # 4096
# Per-tile n sizes: small first tiles shorten the pipeline-fill ramp, big
# tiles keep the output DMAs >= 3 MB for bandwidth.
TILE_SIZES = [256, 256] + [512] * 7
assert sum(TILE_SIZES) == N_LOCAL
NPART = 128

# Engine split for the 25 per-p elementwise ops (b == 0 fast path):
# ScalarE activation / GpSimd tensor_scalar / VectorE tensor_scalar.
SCALAR_PS = set(range(7))
GPSIMD_PS = set(range(7, 11))


def build_bass(with_bias: bool) -> bass.Bass:
    nc = bacc.Bacc(None)

    x_d = nc.dram_tensor("x", (N_LOCAL, WDIM), F32, kind="ExternalInput")
    w_d = nc.dram_tensor("W", (F, P), F32, kind="ExternalInput")
    b_d = nc.dram_tensor("b", (F, 1), F32, kind="ExternalInput")
    o_d = nc.dram_tensor("out", (F, N_LOCAL, P), F32, kind="ExternalOutput")

    with tile.TileContext(nc) as tc:
        with (
            tc.tile_pool(name="const", bufs=1) as constp,
            tc.tile_pool(name="xin", bufs=1) as xinp,
            tc.tile_pool(name="work", bufs=2) as workp,
            tc.tile_pool(name="outp", bufs=3) as outp,
            tc.tile_pool(name="psum", bufs=2, space="PSUM") as psump,
        ):
            # Pull the ACT table load (~1.3 us) off the critical path: a
            # dummy Relu at t=0 makes insert_act_table_loads put it first.
            warm = constp.tile([NPART, 1], F32)
            nc.vector.memset(warm[:], 0.0)
            warm_out = constp.tile([NPART, 1], F32)
            nc.scalar.activation(
                warm_out[:], warm[:], mybir.ActivationFunctionType.Relu
            )

            # W columns and bias replicated on both partition halves; on the
            # SWDGE (gpsimd) path so they don't delay the x loads on HWDGE.
            wcols = constp.tile([NPART, P], F32)
            nc.gpsimd.dma_start(wcols[0:64, :], w_d[:, :])
            nc.gpsimd.dma_start(wcols[64:128, :], w_d[:, :])
            bcol = constp.tile([NPART, 1], F32)
            nc.gpsimd.dma_start(bcol[0:64, :], b_d[:, :])
            nc.gpsimd.dma_start(bcol[64:128, :], b_d[:, :])

            tile_offsets = [sum(TILE_SIZES[:u]) for u in range(len(TILE_SIZES))]

            # --- load x (critical path), one chunk per tile ---
            x_chunks = []
            for u, (n0, tn) in enumerate(zip(tile_offsets, TILE_SIZES)):
                bpt = tn // NPART
                x_ch = xinp.tile(
                    [NPART, bpt, WDIM], F32, name=f"xch{u}", tag=f"xch{u}"
                )
                nc.sync.dma_start(
                    x_ch[:],
                    x_d[n0 : n0 + tn, :].rearrange("(t i) w -> i t w", i=NPART),
                )
                x_chunks.append(x_ch)

            # --- constants ---
            ident = constp.tile([NPART, NPART], F32)
            masks.make_identity(nc, ident[:])

            # Block-diagonal ones: E[k, i] = 1 iff k//64 == i//64.
            e_mat = constp.tile([NPART, NPART], F32)
            nc.gpsimd.memset(e_mat[:], 0.0)
            nc.gpsimd.memset(e_mat[0:64, 0:64], 1.0)
            nc.gpsimd.memset(e_mat[64:128, 64:128], 1.0)

            for u, (n0, tn) in enumerate(zip(tile_offsets, TILE_SIZES)):
                bpt = tn // NPART
                half = tn // 2
                half_blocks = bpt // 2
                out_r = o_d[:, n0 : n0 + tn, :].rearrange(
                    "f (h j) p -> h f j p", h=2, j=half
                )  # (2, 64, half, P)

                # --- transpose n-blocks of 128 into (h, w) layout ---
                xt_ps = [
                    psump.tile(
                        [64, half_blocks, NPART], F32, name=f"xtp{h}", tag=f"xtp{h}"
                    )
                    for h in range(2)
                ]
                for t in range(bpt):
                    h, slot = t // half_blocks, t % half_blocks
                    nc.tensor.transpose(
                        xt_ps[h][:, slot, :], x_chunks[u][:, t, :], ident[:]
                    )
                xt_sb = workp.tile([NPART, half_blocks, NPART], F32, tag="xt_sb")
                nc.vector.tensor_copy(xt_sb[0:64], xt_ps[0][:])
                nc.vector.tensor_copy(xt_sb[64:128], xt_ps[1][:])

                # --- s broadcast: one matmul, K=128 ---
                s_ps = psump.tile([NPART, half], F32, tag="s_ps")
                nc.tensor.matmul(s_ps[:], e_mat[:], xt_sb[:])

                s_sb = workp.tile([NPART, half], F32, tag="s_sb")
                nc.vector.tensor_copy(s_sb[:], s_ps[:])

                # --- 25 per-p elementwise ops ---
                out_t = outp.tile([NPART, half, P], F32, tag="out_t")
                for p in range(P):
                    if with_bias or p in SCALAR_PS:
                        nc.scalar.activation(
                            out_t[:, :, p],
                            s_ps[:],
                            mybir.ActivationFunctionType.Relu,
                            bias=bcol[:, 0:1],
                            scale=wcols[:, p : p + 1],
                        )
                    else:
                        eng = nc.gpsimd if p in GPSIMD_PS else nc.vector
                        eng.tensor_scalar(
                            out_t[:, :, p],
                            s_sb[:],
                            wcols[:, p : p + 1],
                            0.0,
                            mybir.AluOpType.mult,
                            mybir.AluOpType.max,
                        )

                nc.sync.dma_start(out_r, out_t[:])

    nc.compile()
    return nc


_CACHE: dict[bool, bass.Bass] = {}


def _get_bass(with_bias: bool) -> bass.Bass:
    if with_bias not in _CACHE:
        _CACHE[with_bias] = build_bass(with_bias)
    return _CACHE[with_bias]


last_exec_time_ns = None
last_profile = None


def kernel(x, W, b, trace=False, **run_kwargs):
    global last_exec_time_ns, last_profile
    x = np.ascontiguousarray(np.asarray(x, dtype=np.float32)).reshape(N_TOTAL, WDIM)
    wf = np.ascontiguousarray(np.asarray(W, dtype=np.float32)).reshape(F, P)
    bf = np.ascontiguousarray(np.asarray(b, dtype=np.float32)).reshape(F, 1)

    nc = _get_bass(bool(np.any(bf)))

    in_maps = [
        {
            "x": x[m * N_LOCAL : (m + 1) * N_LOCAL],
            "W": wf,
            "b": bf,
        }
        for m in range(N_CORES)
    ]
    res = run_bass_kernel_spmd(
        nc, in_maps, core_ids=list(range(N_CORES)), trace=trace, **run_kwargs
    )
    last_exec_time_ns = res.exec_time_ns
    last_profile = res.profile_json
    outs = [np.asarray(res.results[m]["out"]) for m in range(N_CORES)]
    full = np.concatenate(outs, axis=1)  # (F, N_TOTAL, P)
    return full[None]


# revision 30
# speedup vs baseline: 37502.5076x; 37502.5076x over previous
"""Trainium2 Bass kernel for nn_DeConvAfterDownSampling.

Math (from the reference): with s[n] = sum_w x[b,c,h,w] flattened over
n = (b,c,h), Wf = W.reshape(F, P):

    out[0, f, n, p] = relu(s[n] * Wf[f, p] + b[f])      # (1, F, N, P)

N = 8*64*64 = 32768, F = 64, P = 25.  Output is ~210 MB fp32 while inputs
are ~8 MB, so the kernel is bound by the output HBM write.

Sharding: data-parallel over n across 8 cores (N_LOCAL = 4096 per core);
W and b replicated; no cross-core communication.

Per-core plan (partitions = (h, f) with h in {0,1} stacking two n-halves
so all 128 partitions are used):
  1. One DMA loads x (4096, 64) into SBUF as (128, 32, 64), partition
     i <- row 128*t + i.
  2. PE transposes each (128 n, 64 w) block -> (64 w, 128 n) in PSUM;
     copies assemble xT (128=(h,w), 512 n) in SBUF.
  3. One K=128 matmul with a constant block-diagonal ones matrix E
     (E[(h',w),(h,f)] = (h==h')) reduces over w and broadcasts:
     s_bcast[(h,f), j] = s[tile_base + 512h + j] for every f.
  4. For each p in 0..24 one elementwise op computes
     relu(W[f,p] * s + b[f]) with W[:,p] as per-partition scale and b as
     per-partition bias, writing the (stride 25) p-slice of the output
     tile.  Ops are split across ScalarE (activation, reads PSUM) and
     VectorE (tensor_scalar mult+max, reads an SBUF copy).
  5. One ~6.5 MB DMA per tile writes the (128, 512, 25) tile to HBM; the
     per-partition free layout (n-major, p-minor) is exactly contiguous
     HBM order, so each partition is a single 51.2 KB contiguous chunk.
"""

import numpy as np

import concourse.bass as bass
import concourse.mybir as mybir
from concourse import bacc, masks, tile
from concourse.bass_utils import run_bass_kernel_spmd
from concourse.tile_rust import add_dep_helper

F32 = mybir.dt.float32

N_CORES = 8
B, C, H, WDIM = 8, 64, 64, 64
F, P = 64, 25
N_TOTAL = B * C * H          # 32768
N_LOCAL = N_TOTAL // N_CORES  # 4096
# Per-tile n sizes: small first tiles shorten the pipeline-fill ramp, big
# tiles keep the output DMAs >= 3 MB for bandwidth.
TILE_SIZES = [256, 256] + [512] * 7
assert sum(TILE_SIZES) == N_LOCAL
NPART = 128

# Engine split for the 25 per-p elementwise ops (b == 0 fast path):
# ScalarE activation / GpSimd tensor_scalar / VectorE tensor_scalar.
# Small (ramp) tiles bias away from ScalarE, whose sequencer is slow.
def engine_split(tn):
    if tn <= 256:
        return set(range(6)), set(range(6, 11))
    return set(range(8)), set(range(8, 13))


def build_bass(with_bias: bool) -> bass.Bass:
    nc = bacc.Bacc(None)

    x_d = nc.dram_tensor("x", (N_LOCAL, WDIM), F32, kind="ExternalInput")
    w_d = nc.dram_tensor("W", (F, P), F32, kind="ExternalInput")
    b_d = nc.dram_tensor("b", (F, 1), F32, kind="ExternalInput")
    o_d = nc.dram_tensor("out", (F, N_LOCAL, P), F32, kind="ExternalOutput")

    with tile.TileContext(nc) as tc:
        with (
            tc.tile_pool(name="const", bufs=1) as constp,
            tc.tile_pool(name="xin", bufs=1) as xinp,
            tc.tile_pool(name="work", bufs=2) as workp,
            tc.tile_pool(name="outp", bufs=3) as outp,
            tc.tile_pool(name="psum", bufs=2, space="PSUM") as psump,
        ):
            # Pool builds identity + E first: they gate the PE transposes.
            ident = constp.tile([NPART, NPART], F32)
            masks.make_identity(nc, ident[:])

            # Block-diagonal ones: E[k, i] = 1 iff k//64 == i//64.
            e_mat = constp.tile([NPART, NPART], F32)
            nc.gpsimd.memset(e_mat[:], 0.0)
            nc.gpsimd.memset(e_mat[0:64, 0:64], 1.0)
            nc.gpsimd.memset(e_mat[64:128, 64:128], 1.0)

            # Pull the ACT table load (~1.3 us) off the critical path: a
            # dummy Relu at t=0 makes insert_act_table_loads put it first.
            warm = constp.tile([NPART, 1], F32)
            nc.vector.memset(warm[:], 0.0)
            warm_out = constp.tile([NPART, 1], F32)
            nc.scalar.activation(
                warm_out[:], warm[:], mybir.ActivationFunctionType.Relu
            )

            # W columns replicated on both partition halves, via the ACT
            # HWDGE ring so neither the x loads (SP ring) nor the Pool
            # engine (identity/E) are delayed.
            wcols = constp.tile([NPART, P], F32)
            nc.scalar.dma_start(wcols[0:64, :], w_d[:, :])
            nc.scalar.dma_start(wcols[64:128, :], w_d[:, :])
            if with_bias:
                bcol = constp.tile([NPART, 1], F32)
                nc.scalar.dma_start(bcol[0:64, :], b_d[:, :])
                nc.scalar.dma_start(bcol[64:128, :], b_d[:, :])
                bias_arg = bcol[:, 0:1]
            else:
                # b is all zeros: skip the load, use an immediate bias.
                nc.gpsimd.dma_start(constp.tile([1, 1], F32, name="bjunk")[:],
                                    b_d[0:1, :])  # keep "b" a live input
                bias_arg = 0.0

            tile_offsets = [sum(TILE_SIZES[:u]) for u in range(len(TILE_SIZES))]

            # --- load x (critical path), one chunk per tile ---
            x_chunks = []
            for u, (n0, tn) in enumerate(zip(tile_offsets, TILE_SIZES)):
                bpt = tn // NPART
                x_ch = xinp.tile(
                    [NPART, bpt, WDIM], F32, name=f"xch{u}", tag=f"xch{u}"
                )
                nc.sync.dma_start(
                    x_ch[:],
                    x_d[n0 : n0 + tn, :].rearrange("(t i) w -> i t w", i=NPART),
                )
                x_chunks.append(x_ch)

            # Per-engine chaining of the elementwise ops in program order so
            # the scheduler finishes tile u before starting tile u+1 ops —
            # otherwise cross-tile interleaving delays the first out DMA.
            prev_op = {}

            def chain(key, bi):
                if key in prev_op:
                    add_dep_helper(
                        bi.ins, prev_op[key].ins, sync=False, reason="tile op order"
                    )
                prev_op[key] = bi

            for u, (n0, tn) in enumerate(zip(tile_offsets, TILE_SIZES)):
                bpt = tn // NPART
                half = tn // 2
                half_blocks = bpt // 2
                out_r = o_d[:, n0 : n0 + tn, :].rearrange(
                    "f (h j) p -> h f j p", h=2, j=half
                )  # (2, 64, half, P)

                # --- transpose n-blocks of 128 into (h, w) layout ---
                xt_ps = [
                    psump.tile(
                        [64, half_blocks, NPART], F32, name=f"xtp{h}", tag=f"xtp{h}"
                    )
                    for h in range(2)
                ]
                for t in range(bpt):
                    h, slot = t // half_blocks, t % half_blocks
                    nc.tensor.transpose(
                        xt_ps[h][:, slot, :], x_chunks[u][:, t, :], ident[:]
                    )
                xt_sb = workp.tile([NPART, half_blocks, NPART], F32, tag="xt_sb")
                nc.vector.tensor_copy(xt_sb[0:64], xt_ps[0][:])
                nc.vector.tensor_copy(xt_sb[64:128], xt_ps[1][:])

                # --- s broadcast: one matmul, K=128 ---
                s_ps = psump.tile([NPART, half], F32, tag="s_ps")
                nc.tensor.matmul(s_ps[:], e_mat[:], xt_sb[:])

                s_sb = workp.tile([NPART, half], F32, tag="s_sb")
                nc.vector.tensor_copy(s_sb[:], s_ps[:])

                # --- 25 per-p elementwise ops ---
                out_t = outp.tile([NPART, half, P], F32, tag="out_t")
                scalar_ps, gpsimd_ps = engine_split(tn)
                for p in range(P):
                    if with_bias or p in scalar_ps:
                        bi = nc.scalar.activation(
                            out_t[:, :, p],
                            s_ps[:],
                            mybir.ActivationFunctionType.Relu,
                            bias=bias_arg,
                            scale=wcols[:, p : p + 1],
                        )
                        chain("s", bi)
                    else:
                        gp = p in gpsimd_ps
                        eng = nc.gpsimd if gp else nc.vector
                        bi = eng.tensor_scalar(
                            out_t[:, :, p],
                            s_sb[:],
                            wcols[:, p : p + 1],
                            0.0,
                            mybir.AluOpType.mult,
                            mybir.AluOpType.max,
                        )
                        chain("g" if gp else "v", bi)

                nc.sync.dma_start(out_r, out_t[:])

    nc.compile()
    return nc


_CACHE: dict[bool, bass.Bass] = {}


def _get_bass(with_bias: bool) -> bass.Bass:
    if with_bias not in _CACHE:
        _CACHE[with_bias] = build_bass(with_bias)
    return _CACHE[with_bias]


last_exec_time_ns = None
last_profile = None


def kernel(x, W, b, trace=False, **run_kwargs):
    global last_exec_time_ns, last_profile
    x = np.ascontiguousarray(np.asarray(x, dtype=np.float32)).reshape(N_TOTAL, WDIM)
    wf = np.ascontiguousarray(np.asarray(W, dtype=np.float32)).reshape(F, P)
    bf = np.ascontiguousarray(np.asarray(b, dtype=np.float32)).reshape(F, 1)

    nc = _get_bass(bool(np.any(bf)))

    in_maps = [
        {
            "x": x[m * N_LOCAL : (m + 1) * N_LOCAL],
            "W": wf,
            "b": bf,
        }
        for m in range(N_CORES)
    ]
    res = run_bass_kernel_spmd(
        nc, in_maps, core_ids=list(range(N_CORES)), trace=trace, **run_kwargs
    )
    last_exec_time_ns = res.exec_time_ns
    last_profile = res.profile_json
    outs = [np.asarray(res.results[m]["out"]) for m in range(N_CORES)]
    full = np.concatenate(outs, axis=1)  # (F, N_TOTAL, P)
    return full[None]
